# revision 5
# baseline (speedup 1.0000x reference)
"""Chamfer loss kernel for 8 TRN2 NeuronCores.

Problem: two point clouds target_pc [16384,3], output_pc [16384,3] (f32).
    loss = (sum_i min_j ||o_i - t_j|| + sum_j min_i ||t_j - o_i||) / 1000

Strategy
--------
Each core owns a 2048-row block of output_pc (term 1) and a 2048-row block
of target_pc (term 2) and scans the full opposite cloud. Squared distances
are produced directly by a single K=18 matmul per (row-tile, col-chunk):
coordinates are hi/lo-split into two bf16 parts (x = xh + xm, xm capturing
bits 9-16), and

    |a' - b'|^2 = |a'|^2 + |b'|^2 - 2 sum_d (ah+am)(bh+bm)

is expanded into 18 rank-1 terms (12 cross products + 3-way bf16 splits of
each squared norm). This runs at full PE streaming rate (1 cycle/row, bf16)
while keeping ~2^-16 relative coordinate precision — the f32 PSUM
accumulation returns essentially exact squared distances of points
perturbed by ~1.5e-5.

min_j sqrt(d2) = sqrt(min_j d2), so only the row-min of squared distances
is needed. PSUM evacuation is the bottleneck (1 elem/cycle/partition on
both DVE and ACT), so the row-min is split across engines: per 16384-col
row-tile there are 8 PSUM groups of [128,2048]; 2 are reduced directly by
DVE (fused min-reduce), 6 are evacuated by ScalarE (cast to fp16), then
combined on DVE with fp16 tensor_tensor(min) at 2 elem/cycle and one final
reduce. sqrt+row-sum once per core; host sums the per-partition partials.
"""

import sys

for _p in ("/opt/trn_rl_repo",):
    if _p not in sys.path:
        sys.path.insert(0, _p)

import ml_dtypes
import numpy as np

import concourse.bass as bass
from concourse import bacc, mybir, tile
from concourse.bass_utils import run_bass_kernel_spmd

N = 16384          # points per cloud
NCORES = 8
ROWS = N // NCORES     # 2048 rows of the "query" cloud per core
PT = 128               # query rows per partition tile
NT = ROWS // PT        # 16 partition tiles per term
CHUNK = 512            # db columns per matmul (one PSUM bank)
GROUP = 4              # chunks per PSUM group ([128, 2048] = 4 banks)
GCOLS = CHUNK * GROUP
NG = N // GCOLS        # 8 groups per row-tile
NDIRECT = 2            # groups min-reduced directly from PSUM by DVE
CAND = NDIRECT + 1     # min candidates per row-tile (direct + tree)
KR = 18                # rank-1 terms (matmul contraction dim)

F32 = mybir.dt.float32
FP16 = mybir.dt.float16
BF16 = mybir.dt.bfloat16
NPBF16 = np.dtype(ml_dtypes.bfloat16)


def _build_program():
    nc = bacc.Bacc("TRN2", target_bir_lowering=False, debug=False,
                   num_devices=NCORES)

    lq1 = nc.dram_tensor("lq1", [KR, ROWS], BF16, kind="ExternalInput").ap()
    db1 = nc.dram_tensor("db1", [KR, N], BF16, kind="ExternalInput").ap()
    lq2 = nc.dram_tensor("lq2", [KR, ROWS], BF16, kind="ExternalInput").ap()
    db2 = nc.dram_tensor("db2", [KR, N], BF16, kind="ExternalInput").ap()
    out = nc.dram_tensor("out", [128, 1], F32, kind="ExternalOutput").ap()

    with tile.TileContext(nc) as tc:
        _chamfer(tc, out, lq1, db1, lq2, db2)
    nc.compile()
    return nc


def _chamfer(tc, out, lq1, db1, lq2, db2):
    nc = tc.nc
    from contextlib import ExitStack

    with ExitStack() as ctx:
        singles = ctx.enter_context(tc.tile_pool(name="singles", bufs=1))
        psum_pool = ctx.enter_context(
            tc.tile_pool(name="psum", bufs=2, space="PSUM"))
        evac = ctx.enter_context(tc.tile_pool(name="evac", bufs=8))
        treep = ctx.enter_context(tc.tile_pool(name="treep", bufs=4))
        small = ctx.enter_context(tc.tile_pool(name="small", bufs=1))

        # --- load inputs (one-time) -------------------------------------
        sb_lq1 = singles.tile([KR, ROWS], BF16, tag="lq1")
        nc.sync.dma_start(sb_lq1[:], lq1[:])
        sb_db1 = singles.tile([KR, N], BF16, tag="db1")
        nc.sync.dma_start(sb_db1[:], db1[:])
        sb_lq2 = singles.tile([KR, ROWS], BF16, tag="lq2")
        nc.sync.dma_start(sb_lq2[:], lq2[:])
        sb_db2 = singles.tile([KR, N], BF16, tag="db2")
        nc.sync.dma_start(sb_db2[:], db2[:])

        # per-(term,row-tile) min candidates
        pm = small.tile([128, 2 * NT * CAND], F32, tag="pm")

        for term, (sb_lq, sb_db) in enumerate(((sb_lq1, sb_db1),
                                               (sb_lq2, sb_db2))):
            for t in range(NT):
                lhsT = sb_lq[:, t * PT:(t + 1) * PT]
                cbase = (term * NT + t) * CAND
                evs = []
                for g in range(NG):
                    pg = psum_pool.tile([128, GCOLS], F32, tag="pg")
                    for c in range(GROUP):
                        col = g * GCOLS + c * CHUNK
                        nc.tensor.matmul(
                            pg[:, c * CHUNK:(c + 1) * CHUNK],
                            lhsT,
                            sb_db[:, col:col + CHUNK],
                            start=True, stop=True,
                        )
                    if g < NDIRECT:
                        nc.vector.tensor_reduce(
                            out=pm[:, cbase + g:cbase + g + 1],
                            in_=pg[:],
                            axis=mybir.AxisListType.X,
                            op=mybir.AluOpType.min,
                        )
                    else:
                        ev = evac.tile([128, GCOLS], FP16, tag="ev")
                        nc.scalar.copy(ev[:], pg[:])
                        evs.append(ev)
                # fp16 pairwise-min tree over the 6 evacuated groups
                while len(evs) > 1:
                    nxt = []
                    for i in range(0, len(evs) - 1, 2):
                        x = treep.tile([128, GCOLS], FP16, tag="tx")
                        nc.vector.tensor_tensor(
                            out=x[:], in0=evs[i][:], in1=evs[i + 1][:],
                            op=mybir.AluOpType.min)
                        nxt.append(x)
                    if len(evs) % 2:
                        nxt.append(evs[-1])
                    evs = nxt
                nc.vector.tensor_reduce(
                    out=pm[:, cbase + NDIRECT:cbase + NDIRECT + 1],
                    in_=evs[0][:],
                    axis=mybir.AxisListType.X,
                    op=mybir.AluOpType.min,
                )

        # --- epilogue ---------------------------------------------------
        # row-min over the CAND candidates -> [128, 32] per-row sq dist
        mall = small.tile([128, 2 * NT], F32, tag="mall")
        nc.vector.tensor_reduce(
            out=mall[:],
            in_=pm.rearrange("p (k r) -> p k r", r=CAND),
            axis=mybir.AxisListType.X,
            op=mybir.AluOpType.min,
        )
        # clamp tiny negatives from f32 cancellation, then sqrt + row sum
        mclamp = small.tile([128, 2 * NT], F32, tag="mclamp")
        nc.vector.tensor_scalar(
            out=mclamp[:], in0=mall[:], scalar1=0.0, scalar2=None,
            op0=mybir.AluOpType.max,
        )
        sq = small.tile([128, 2 * NT], F32, tag="sq")
        ssum = small.tile([128, 1], F32, tag="ssum")
        nc.scalar.activation(
            out=sq[:], in_=mclamp[:],
            func=mybir.ActivationFunctionType.Sqrt,
            accum_out=ssum[:],
        )
        nc.sync.dma_start(out[:], ssum[:])


_CACHED_NC = None


def _get_nc():
    global _CACHED_NC
    if _CACHED_NC is None:
        _CACHED_NC = _build_program()
    return _CACHED_NC


def _split2(x32):
    """f32 [n,3] -> (hi, lo) bf16 parts with x ~= hi + lo (~2^-16 resid)."""
    h = x32.astype(NPBF16)
    m = (x32 - h.astype(np.float32)).astype(NPBF16)
    return h, m


def _split3(v64):
    """f64 [n] -> 3 bf16 parts summing to v (~2^-24 resid)."""
    p0 = v64.astype(NPBF16)
    r = v64 - p0.astype(np.float64)
    p1 = r.astype(NPBF16)
    r = r - p1.astype(np.float64)
    p2 = r.astype(NPBF16)
    return p0, p1, p2


_PARTS = ((0, 0), (0, 1), (1, 0), (1, 1))  # (query part, db part) pairing


def _pack_query(a):
    """[n,3] f32 -> [18,n] bf16 lhsT rows: -2*a_p[dim] | 1 | sq_a parts."""
    a32 = np.asarray(a, np.float32)
    n = a32.shape[0]
    h, m = _split2(a32)
    parts = (h, m)
    ar = h.astype(np.float64) + m.astype(np.float64)
    sq = (ar * ar).sum(axis=1)
    s0, s1, s2 = _split3(sq)
    q = np.empty((KR, n), NPBF16)
    for dim in range(3):
        for j, (pq, _) in enumerate(_PARTS):
            q[dim * 4 + j] = (
                -2.0 * parts[pq][:, dim].astype(np.float32)).astype(NPBF16)
    q[12] = 1.0
    q[13] = 1.0
    q[14] = 1.0
    q[15], q[16], q[17] = s0, s1, s2
    return np.ascontiguousarray(q)


def _pack_db(b):
    """[n,3] f32 -> [18,n] bf16 rhs rows: b_q[dim] | sq_b parts | 1."""
    b32 = np.asarray(b, np.float32)
    n = b32.shape[0]
    h, m = _split2(b32)
    parts = (h, m)
    br = h.astype(np.float64) + m.astype(np.float64)
    sq = (br * br).sum(axis=1)
    s0, s1, s2 = _split3(sq)
    d = np.empty((KR, n), NPBF16)
    for dim in range(3):
        for j, (_, pd) in enumerate(_PARTS):
            d[dim * 4 + j] = parts[pd][:, dim]
    d[12], d[13], d[14] = s0, s1, s2
    d[15] = 1.0
    d[16] = 1.0
    d[17] = 1.0
    return np.ascontiguousarray(d)


def _make_in_maps(target_pc, output_pc):
    q1 = _pack_query(output_pc)   # term 1: queries = output_pc
    d1 = _pack_db(target_pc)
    q2 = _pack_query(target_pc)   # term 2: queries = target_pc
    d2 = _pack_db(output_pc)
    in_maps = []
    for c in range(NCORES):
        sl = slice(c * ROWS, (c + 1) * ROWS)
        in_maps.append({
            "lq1": np.ascontiguousarray(q1[:, sl]),
            "db1": d1,
            "lq2": np.ascontiguousarray(q2[:, sl]),
            "db2": d2,
        })
    return in_maps


def kernel(target_pc, output_pc):
    target_pc = np.asarray(target_pc, np.float32)
    output_pc = np.asarray(output_pc, np.float32)

    in_maps = _make_in_maps(target_pc, output_pc)
    nc = _get_nc()
    res = run_bass_kernel_spmd(nc, in_maps, list(range(NCORES)))
    total = np.float64(0.0)
    for c in range(NCORES):
        total += np.float64(res.results[c]["out"][:, 0].sum())
    return np.float32(total / 1000.0)
